# revision 17
# baseline (speedup 1.0000x reference)
"""Deformable-conv layer kernel for 8 Trainium2 NeuronCores (Bass/Tile), v2.

kernel(**inputs): full inputs -> full output [2,48,48,24] f32.
Data parallel over (batch, H/4) -> 8 shards of 576 pixels.

Per core (576 pixels s, 216 sample-channels k = 9 taps x 24 groups):
  G[y, k*48+c] = sum_ch I'[y*48+c, ch] * k4[k, ch]       (PE, fp32r)
  rep = d[r, s]: PE "broadcast" matmul, contraction over 5 coordinate
    component rows (yi, yf, xi, xf, 1) -> d[r,s] = coord[s] - r for
    y rows 0-47 / x rows 48-95 (integer/fraction split keeps fp32r exact)
  tnt = +-tent(d) via Abs+Relu (Act) or Abs + tensor_scalar min (DVE/Pool)
  T_k  = G_k^T @ tnt[y rows]  -> [48c x s]               (PE, fp32r)
  P_k  = T_k * tnt[x rows]                               (DVE/Pool)
  acc[2u] += ones2^T @ P-pair  (u-paired partition sum, 9-tap PSUM accum)
tent sign flips per-iteration depending on engine (Act: +, min-trick: -);
P = (+-ty)*(+-tx) is always positive. The y==47 / x==47 clip corner
(reference weights all zero) is handled with a +1e6 coordinate offset.
"""

import sys

for _p in ("/opt/trn_rl_repo",):
    if _p not in sys.path:
        sys.path.insert(0, _p)

import numpy as np

B, H, W, C = 2, 48, 48, 32
U = 24
KH = KW = 3
PAD = 1
K = KH * KW * U          # 216
NCORES = 8
HLOC = H // 4            # 12
S = HLOC * W             # 576
BIG = 1.0e6
SC = 288                 # psum chunk (2 chunks of 288 = 576)

_PROG = {}


# per-iteration engine assignment knobs (keyed on emission index so the
# mix is uniform within every pair)
def _t_engine(i):
    r = i % 20
    return "act" if r in (1, 11) else "pool"


def _m_engine(i):
    return "dve"


def _base_grids():
    hh = np.arange(H)[:, None, None, None] + np.arange(KH)[None, None, :, None] - PAD
    ww = np.arange(W)[None, :, None, None] + np.arange(KW)[None, None, None, :] - PAD
    hh = np.broadcast_to(hh, (H, W, KH, KW))
    ww = np.broadcast_to(ww, (H, W, KH, KW))
    valid = (hh >= 0) & (hh < H) & (ww >= 0) & (ww < W)
    yb = np.where(valid, hh, 0).reshape(H, W, 9).astype(np.float32)
    xb = np.where(valid, ww, 0).reshape(H, W, 9).astype(np.float32)
    return yb, xb


def _build_program():
    import concourse.mybir as mybir
    import concourse.tile as tile
    from concourse import bacc

    f32 = mybir.dt.float32
    f32r = mybir.dt.float32r
    Alu = mybir.AluOpType
    Act = mybir.ActivationFunctionType

    nc = bacc.Bacc("TRN2", target_bir_lowering=False, debug=False)

    d_mtw = nc.declare_dram_parameter("mtw", [C, H * W], f32, isOutput=False)
    d_kct = nc.declare_dram_parameter("kct", [C, 256], f32, isOutput=False)
    d_cyx = [
        nc.declare_dram_parameter(f"cyx{t}", [5, 18 * S], f32, isOutput=False)
        for t in range(U // 2)
    ]
    d_sel = nc.declare_dram_parameter("sel5", [5, 113], f32, isOutput=False)
    d_gsum = nc.declare_dram_parameter("gsumneg", [1, K * 48], f32, isOutput=False)
    d_ones2 = nc.declare_dram_parameter("ones2", [112, 2], f32, isOutput=False)
    d_bias = nc.declare_dram_parameter("biasr", [2, U // 2], f32, isOutput=False)
    d_out = nc.declare_dram_parameter("out", [2, (U // 2) * S], f32, isOutput=True)

    with tile.TileContext(nc) as tc:
        with (
            tc.tile_pool(name="persist", bufs=1) as pp,
            tc.tile_pool(name="coord", bufs=2) as cp,
            tc.tile_pool(name="work", bufs=6) as wp,
            tc.tile_pool(name="psR", bufs=2, space="PSUM") as psR,
            tc.tile_pool(name="psA", bufs=2, space="PSUM") as psA,
            tc.tile_pool(name="psB", bufs=1, space="PSUM") as psB,
        ):
            # ---- constant loads + fp32r-rounding copies for PE operands ----
            mtw0 = pp.tile([C, H * W], f32, tag="mtw0", name="mtw0")
            nc.sync.dma_start(out=mtw0[:], in_=d_mtw[:])
            mtw = pp.tile([C, H * W], f32, tag="mtw", name="mtw")
            nc.vector.tensor_copy(out=mtw[:].bitcast(f32r), in_=mtw0[:])
            kct0 = pp.tile([C, 256], f32, tag="kct0", name="kct0")
            nc.sync.dma_start(out=kct0[:], in_=d_kct[:])
            kct = pp.tile([C, 256], f32, tag="kct", name="kct")
            nc.vector.tensor_copy(out=kct[:].bitcast(f32r), in_=kct0[:])
            sel0 = pp.tile([5, 113], f32, tag="sel0", name="sel0")
            nc.sync.dma_start(out=sel0[:], in_=d_sel[:])
            sel5 = pp.tile([5, 113], f32, tag="sel5", name="sel5")
            nc.vector.tensor_copy(out=sel5[:].bitcast(f32r), in_=sel0[:])
            on0 = pp.tile([112, 2], f32, tag="on0", name="on0")
            nc.sync.dma_start(out=on0[:], in_=d_ones2[:])
            ones2 = pp.tile([112, 2], f32, tag="ones2", name="ones2")
            nc.vector.tensor_copy(out=ones2[:].bitcast(f32r), in_=on0[:])
            biasr = pp.tile([2, U // 2], f32, tag="biasr", name="biasr")
            nc.sync.dma_start(out=biasr[:], in_=d_bias[:])

            # ---- G build: G[y, k*48+c]; row 48 = -colsum(G) ----
            g48 = pp.tile([49, K * 48], f32, tag="g48", name="g48")
            nc.sync.dma_start(
                out=g48[48:49, :].bitcast(f32r), in_=d_gsum[:].bitcast(f32r)
            )
            for c in range(0, 48, 2):
                gp = psA.tile([48, 2, 256], f32, tag="ta", name=f"gp{c}")
                for j in range(2):
                    nc.tensor.matmul(
                        out=gp[:, j, 0:256],
                        lhsT=mtw[:, c + j :: 48].bitcast(f32r),
                        rhs=kct[:].bitcast(f32r),
                        start=True, stop=True,
                    )
                # one strided copy writes both columns c, c+1 of every k block
                dst = g48[0:48, c : c + K * 48]
                dst = dst.reshape([48, K, 48])[:, :, 0:2]
                if (c // 2) % 2 == 0:
                    nc.scalar.copy(out=dst.bitcast(f32r), in_=gp[:, :, 0:K])
                else:
                    nc.vector.tensor_copy(out=dst.bitcast(f32r),
                                          in_=gp[:, :, 0:K])

            # ---- main loop: 12 u-pairs x 9 taps x 2 in-pair ----
            out24 = pp.tile([2, (U // 2) * S], f32, tag="out24", name="out24")
            CH = ((0, 0), (SC, 512))      # (data offset, acc offset)
            for p in range(U // 2):
                cyx = cp.tile([5, 18 * S], f32r, tag="cyx", name=f"cyx{p}")
                nc.sync.dma_start(out=cyx[:], in_=d_cyx[p][:].bitcast(f32r))
                acc = psB.tile([2, 1024], f32, tag="acc", name=f"acc_{p}")
                for ij in range(9):
                    pt2 = wp.tile([112, S], f32, tag="pt2", name=f"pt2_{p}_{ij}")
                    if p * 9 + ij < 6:
                        nc.scalar.memzero(pt2[32:64, :])
                    for uu in range(2):
                        k = ij * U + 2 * p + uu
                        kb = (ij * 2 + uu) * S
                        idx = p * 18 + ij * 2 + uu
                        te = _t_engine(idx)

                        tnt = wp.tile([113, S], f32, tag="tnt", name=f"tnt_{k}")
                        rep = psR.tile([113, 2, 512], f32, tag="rep",
                                       name=f"rep_{k}")
                        for ci, (lo, _) in enumerate(CH):
                            nc.tensor.matmul(
                                out=rep[:, ci, 0:SC],
                                lhsT=sel5[:].bitcast(f32r),
                                rhs=cyx[:, kb + lo : kb + lo + SC],
                                start=True, stop=True,
                            )
                        # |d| in one op; tent (act) / -tent (pool) below
                        nc.scalar.activation(
                            out=tnt[:, 0:S].bitcast(f32r),
                            in_=rep[:, :, 0:SC],
                            func=Act.Abs, bias=0.0, scale=1.0,
                        )
                        if te == "act":
                            nc.scalar.activation(
                                out=tnt[:].bitcast(f32r), in_=tnt[:],
                                func=Act.Relu, bias=1.0, scale=-1.0,
                            )
                        elif te == "pool":
                            # -tent = min(|d|-1, 0); row 48 -> 0, and the
                            # two sign flips cancel in P (SBUF-only op)
                            nc.gpsimd.tensor_scalar(
                                out=tnt[:].bitcast(f32r), in0=tnt[:],
                                scalar1=1.0, scalar2=0.0,
                                op0=Alu.subtract, op1=Alu.min,
                            )

                        for lo, _ in CH:
                            ta = psA.tile([48, SC], f32, tag="ta",
                                          name=f"ta_{k}_{lo}")
                            nc.tensor.matmul(
                                out=ta[:],
                                lhsT=g48[:, 48 * k : 48 * (k + 1)].bitcast(f32r),
                                rhs=tnt[0:49, lo : lo + SC].bitcast(f32r),
                                start=True, stop=True,
                            )
                            dst = pt2[64 * uu : 64 * uu + 48,
                                      lo : lo + SC].bitcast(f32r)
                            nc.vector.tensor_tensor(
                                out=dst, in0=ta[:],
                                in1=tnt[64:112, lo : lo + SC],
                                op=Alu.mult,
                            )
                    for lo, ao in CH:
                        nc.tensor.matmul(
                            out=acc[:, ao : ao + SC],
                            lhsT=ones2[:].bitcast(f32r),
                            rhs=pt2[:, lo : lo + SC].bitcast(f32r),
                            start=(ij == 0), stop=(ij == 8),
                            skip_group_check=True,
                        )
                for lo, ao in CH:
                    nc.scalar.activation(
                        out=out24[:, p * S + lo : p * S + lo + SC],
                        in_=acc[:, ao : ao + SC],
                        func=Act.Identity, bias=biasr[:, p : p + 1],
                        scale=1.0,
                    )
            nc.sync.dma_start(out=d_out[:], in_=out24[:])

    nc.compile()
    return nc


def kernel(inputs, offset, kernel, bias):
    from concourse.bass_utils import run_bass_kernel_spmd

    inputs = np.asarray(inputs, np.float32)
    offset = np.asarray(offset, np.float32)
    kernel = np.asarray(kernel, np.float32)
    bias = np.asarray(bias, np.float32)

    if "nc" not in _PROG:
        _PROG["nc"] = _build_program()
    nc = _PROG["nc"]

    yb9, xb9 = _base_grids()
    k4 = kernel.reshape(9, U, C).reshape(K, C)
    kct = np.zeros((C, 256), np.float32)
    kct[:, 0:K] = k4.T

    sel5 = np.zeros((5, 113), np.float32)
    r = np.arange(113)
    sel5[0, :] = (r < 48)                  # yi
    sel5[1, :] = (r < 48)                  # yf
    sel5[2, :] = (r >= 64)                 # xi
    sel5[3, :] = (r >= 64)                 # xf
    sel5[4, :] = -np.where(r < 48, r, np.maximum(r - 64, 0)).astype(np.float32)
    sel5[4, 48] = 9.0                      # d[48] = 9 -> row48: min-path 1, tent-path 0
    sel5[4, 49:64] = 9.0                   # unused rows, keep tents there 0/1
    ones2 = np.zeros((112, 2), np.float32)
    ones2[0:48, 0] = 1.0
    ones2[64:112, 1] = 1.0
    biasr = np.ascontiguousarray(bias.reshape(U // 2, 2).T)

    in_maps = []
    for core in range(NCORES):
        bb, hc = divmod(core, 4)
        h0 = hc * HLOC
        xpad = np.pad(inputs[bb], ((PAD, PAD), (PAD, PAD), (0, 0)))[:H, :W]
        mtw = np.ascontiguousarray(xpad.reshape(H * W, C).T)      # [32, 2304]
        # -colsum of G per (k, c): G[r, 48k+c] = sum_ch I'[r,c,ch] k4[k,ch]
        colsum = xpad.sum(axis=0).astype(np.float32)              # [48, 32]
        gsum = k4 @ colsum.T                                      # [216k, 48c]
        gsumneg = np.ascontiguousarray(-gsum.reshape(1, K * 48))
        osl = offset[bb, h0 : h0 + HLOC].reshape(S, K, 2)
        yc = yb9[h0 : h0 + HLOC].reshape(S, 9)
        xc = xb9[h0 : h0 + HLOC].reshape(S, 9)
        y = np.clip(np.repeat(yc, U, axis=1) + osl[:, :, 0], 0.0, 47.0)  # [S,K]
        x = np.clip(np.repeat(xc, U, axis=1) + osl[:, :, 1], 0.0, 47.0)
        y = y + BIG * (y == 47.0)
        x = x + BIG * (x == 47.0)
        yi = np.floor(y)
        xi = np.floor(x)
        yf = (y - yi).astype(np.float32)
        xf = (x - xi).astype(np.float32)
        comp = (yi.T.astype(np.float32), yf.T, xi.T.astype(np.float32), xf.T)
        im = {"mtw": mtw, "kct": kct, "sel5": sel5, "ones2": ones2,
              "biasr": biasr, "gsumneg": gsumneg}
        for t in range(U // 2):
            blk = np.ones((5, 18 * S), np.float32)
            for ij in range(9):
                for uu in range(2):
                    k = ij * U + 2 * t + uu
                    col = (ij * 2 + uu) * S
                    for rr in range(4):
                        blk[rr, col : col + S] = comp[rr][k]
            im[f"cyx{t}"] = blk
        in_maps.append(im)

    import os as _os
    _trace = bool(int(_os.environ.get("KERNEL_TRACE", "0")))
    res = run_bass_kernel_spmd(
        nc, in_maps, list(range(NCORES)), trace=_trace)
    _PROG["last_results"] = res

    out = np.empty((B, H, W, U), np.float32)
    for core in range(NCORES):
        bb, hc = divmod(core, 4)
        h0 = hc * HLOC
        o = res.results[core]["out"].reshape(2, U // 2, HLOC, W)
        out[bb, h0 : h0 + HLOC] = o.transpose(2, 3, 1, 0).reshape(HLOC, W, U)
    return out
